# revision 1
# baseline (speedup 1.0000x reference)
"""CrossNetMix (moe_routing) Trainium2 Bass kernel — v2 (bf16 + pipelined PE).

Math per layer (B=16384, D=1024, R=64, E=4, L=3):
    g  = softmax(xl @ gates_w.T)                   # [B, E]
    t1 = tanh(einsum('erd,bd->ber', V, xl))        # [B, E, R]
    t2 = tanh(einsum('ers,bes->ber', C, t1))       # [B, E, R]
    d  = einsum('edr,ber->bed', U, t2) + bias      # [B, E, D]
    xl = xl + x0 * sum_e g_e * d_e                 # gated combine + residual

On-chip layout: d on partitions, b on free dim; batch sharded 8 ways
(B_c = 2048/core, 4 b-tiles of 512). All matmul data is bf16 (1 cyc/row
on PE, fp32 PSUM accumulation); the ~0.1% bf16 rounding is far inside
the 2e-2 accuracy gate.

Engine balance (the v1 kernel was DVE-bound at 73% with Pool idle):
  - PE  : gates(8) V(16) sum(1) C(2) g-replication(2) U(16) matmuls/slot.
  - ACT : exp + 4x tanh only (bias-evac eliminated).
  - DVE : softmax recip/normalize, gate-scaling muls, part of the fused
          epilogue, and all residual adds (bf16 SBUF ops run in the DVE
          2x perf mode: 327 ns vs 594 ns for fp32).
  - Pool: the other part of the fused epilogue (was 100% idle in v1).
Epilogue per d-tile is ONE scalar_tensor_tensor
    tm3 = (ups_PSUM + bias_col) * x0        (op0=add, op1=mult)
plus one bf16 add xl += tm3 — replacing v1's ACT bias-evac + DVE mul +
DVE add. x0 stays resident in SBUF (v1 re-streamed 16 MB/core of x0
from DRAM every non-first layer).

PE stream is software-pipelined: slot i runs gates/V/sum/C/replication
of tile i but the U-pass of tile i-1, so PE never waits on the
exp->recip->normalize->replicate or tanh chains. PSUM banks: 1 gates +
2 V/C + 4 U + 1 replication = 8.

DMA issue time on the sync sequencer is ~1.3 us per transfer, so
transfers are coalesced: all per-layer stationaries ride in two packed
[128, n] blobs, x/x0 load in one DMA per b-tile, and each output slot
stores via two packed DMAs (v1 issued 148 transfers; v2 issues 22).
"""

import numpy as np
import ml_dtypes

import concourse.bass as bass
import concourse.tile as tile
from concourse import bacc, mybir
from concourse.bass_utils import run_bass_kernel_spmd

B, D, R, E, L = 16384, 1024, 64, 4, 3
NCORES = 8
BC = B // NCORES          # 2048 rows per core
NBT = 4                   # b tiles of 512
BT = BC // NBT
NK = D // 128             # 8 k/d tiles
NSLOT = L * NBT

BF16 = mybir.dt.bfloat16
F32 = mybir.dt.float32
AF = mybir.ActivationFunctionType
AO = mybir.AluOpType

_cache = {}

# Per-d-tile epilogue path. GPSIMD cannot touch PSUM, so the PSUM read is
# always on DVE (fused scalar_tensor_tensor) or ACT (bias-evac); the
# remaining mul/add ops are spread so no engine exceeds ~72% busy:
#   a: DVE stt  -> DVE add
#   b: DVE stt  -> Pool add
#   d: ACT evac -> DVE mul -> Pool add
EPI_PATH = {0: "a", 1: "d", 2: "b", 3: "d", 4: "a", 5: "d", 6: "b", 7: "d"}

# packed-parameter blob column offsets (bf16 elements)
GT_O = 0                      # gates_w.T chunks      [NK*E]
CB0_O = GT_O + NK * E         # C block-diag, l=0     [2*128]
VT0_O = CB0_O + 2 * 128       # V.T, l=0              [NK*256]
PK0_W = VT0_O + NK * 256      # (U.T of l=0 rides in its own transfer)
LBLK = 2 * 128 + NK * 256 + 2 * 1024   # per-layer block in pk12 (l=1,2)
PK12_W = 2 * LBLK


def _build(repeat=1, bench=False):
    key = (repeat, bench)
    if key in _cache:
        return _cache[key]
    nc = bacc.Bacc("TRN2", target_bir_lowering=False, debug=False)
    if bench:
        # Timing-only build: no real I/O transfers — all data tensors live
        # in internal DRAM (garbage values; engine timing is data-blind).
        dummy_in = nc.dram_tensor("dummy_in", [1, 1], F32, kind="ExternalInput")
        dummy_out = nc.dram_tensor("dummy_out", [1, 1], F32, kind="ExternalOutput")
        mk = lambda name, shape, dt: nc.dram_tensor(name, shape, dt)
    else:
        mk = lambda name, shape, dt: nc.dram_tensor(name, shape, dt, kind="ExternalInput")
    XT = mk("XT", [128, NK, BC], BF16)
    PK0 = mk("PK0", [128, PK0_W], BF16)
    UT0 = mk("UT0", [128, 2 * 1024], BF16)
    PK12 = mk("PK12", [128, PK12_W], BF16)
    RR = mk("RR", [E, E + 2 * 128], BF16)
    BTb = mk("BTb", [128, L * NK], F32)
    if bench:
        OT = nc.dram_tensor("OT", [128, NK, BC], BF16)
    else:
        OT = nc.dram_tensor("OT", [128, NK, BC], BF16, kind="ExternalOutput")

    with tile.TileContext(nc) as tc:
        xlT = nc.alloc_sbuf_tensor("xlT", [128, NK, BC], BF16)
        x0T = nc.alloc_sbuf_tensor("x0T", [128, NK, BC], BF16)
        pk0 = nc.alloc_sbuf_tensor("pk0", [128, PK0_W], BF16)
        ut0 = nc.alloc_sbuf_tensor("ut0", [128, 2 * 1024], BF16)
        pk12 = nc.alloc_sbuf_tensor("pk12", [128, PK12_W], BF16)
        rr = nc.alloc_sbuf_tensor("rr", [E, E + 2 * 128], BF16)
        bt_sb = nc.alloc_sbuf_tensor("bt_sb", [128, L * NK], F32)

        def gt_ap(k):
            return pk0.ap()[:, GT_O + k * E : GT_O + (k + 1) * E]

        def cb_ap(l, p):
            if l == 0:
                o = CB0_O + p * 128
                return pk0.ap()[:, o : o + 128]
            o = (l - 1) * LBLK + p * 128
            return pk12.ap()[:, o : o + 128]

        def vt_ap(l, k, p):
            if l == 0:
                o = VT0_O + k * 256 + p * 128
                return pk0.ap()[:, o : o + 128]
            o = (l - 1) * LBLK + 2 * 128 + k * 256 + p * 128
            return pk12.ap()[:, o : o + 128]

        def ut_ap(l, p, dt):
            if l == 0:
                o = p * 1024 + dt * 128
                return ut0.ap()[:, o : o + 128]
            o = (l - 1) * LBLK + 2 * 128 + NK * 256 + p * 1024 + dt * 128
            return pk12.ap()[:, o : o + 128]

        rps_ap = rr.ap()[:, 0:E]

        def rpr_ap(p):
            return rr.ap()[:, E + p * 128 : E + (p + 1) * 128]

        # --- loads, ordered so slot-0 compute starts ~4 us in and every
        # later tensor lands well before its first use ---
        bs0 = bass.ts(0, BT)
        nc.sync.dma_start(pk0.ap()[:, :CB0_O], PK0.ap()[:, :CB0_O])
        nc.sync.dma_start(
            xlT.ap()[:, 0 : NK // 2, bs0], XT.ap()[:, 0 : NK // 2, bs0]
        )
        nc.sync.dma_start(
            xlT.ap()[:, NK // 2 : NK, bs0], XT.ap()[:, NK // 2 : NK, bs0]
        )
        nc.sync.dma_start(pk0.ap()[:, CB0_O:], PK0.ap()[:, CB0_O:])
        nc.sync.dma_start(x0T.ap()[:, :, bs0], XT.ap()[:, :, bs0])
        nc.sync.dma_start(rr.ap(), RR.ap())
        nc.sync.dma_start(bt_sb.ap(), BTb.ap())
        nc.sync.dma_start(ut0.ap(), UT0.ap())
        for bt in range(1, NBT):
            bs = bass.ts(bt, BT)
            nc.sync.dma_start(xlT.ap()[:, :, bs], XT.ap()[:, :, bs])
            nc.sync.dma_start(x0T.ap()[:, :, bs], XT.ap()[:, :, bs])
        nc.sync.dma_start(pk12.ap(), PK12.ap())

        from contextlib import ExitStack

        ctx = ExitStack()
        # PSUM budget is 8 banks ([128,512]f32 = 1 bank).
        ps_gs = ctx.enter_context(tc.tile_pool(name="ps_gs", bufs=1, space="PSUM"))
        ps_vc = ctx.enter_context(tc.tile_pool(name="ps_vc", bufs=2, space="PSUM"))
        ps_u = ctx.enter_context(tc.tile_pool(name="ps_u", bufs=4, space="PSUM"))
        ps_e = ctx.enter_context(tc.tile_pool(name="ps_e", bufs=1, space="PSUM"))
        sb_t1 = ctx.enter_context(tc.tile_pool(name="sb_t1", bufs=3))
        sb_t2 = ctx.enter_context(tc.tile_pool(name="sb_t2", bufs=3))
        sb_t2s = ctx.enter_context(tc.tile_pool(name="sb_t2s", bufs=2))
        sb_e4 = ctx.enter_context(tc.tile_pool(name="sb_e4", bufs=2))
        sb_g4 = ctx.enter_context(tc.tile_pool(name="sb_g4", bufs=2))
        sb_rs = ctx.enter_context(tc.tile_pool(name="sb_rs", bufs=2))
        sb_tm = ctx.enter_context(tc.tile_pool(name="sb_tm", bufs=4))
        sb_ot = ctx.enter_context(tc.tile_pool(name="sb_ot", bufs=2))

        def emit_U_half(j, t2s, half, drain=False):
            """U-pass matmuls for slot j, d-tiles [half*4, half*4+4) (PE
            stream). In the pipeline drain the V/C banks are idle, so borrow
            them to avoid PSUM-recycle stalls while the epilogue engines
            catch up."""
            l, bt = divmod(j, NBT)
            ups_tiles = []
            for dt in range(half * (NK // 2), (half + 1) * (NK // 2)):
                if drain and dt % 2 == 1:
                    ups = ps_vc.tile([128, BT], F32, tag="vc")
                else:
                    ups = ps_u.tile([128, BT], F32, tag="u")
                nc.tensor.matmul(
                    ups[:], ut_ap(l, 0, dt), t2s[0][:], start=True, stop=False
                )
                nc.tensor.matmul(
                    ups[:], ut_ap(l, 1, dt), t2s[1][:], start=False, stop=True
                )
                ups_tiles.append(ups)
            return ups_tiles

        def emit_epilogue_half(j, ups_tiles, half, ot):
            """(PSUM+bias)*x0 then residual add for slot j's d-tiles
            [half*4, half*4+4), spread across DVE/ACT/Pool per EPI_PATH."""
            l, bt = divmod(j, NBT)
            bs = bass.ts(bt, BT)
            for i, dt in enumerate(range(half * (NK // 2), (half + 1) * (NK // 2))):
                path = EPI_PATH[dt]
                bias_col = bt_sb.ap()[:, l * NK + dt : l * NK + dt + 1]
                tm3 = sb_tm.tile([128, BT], BF16, tag="tm")
                if path == "d":
                    tm2 = sb_tm.tile([128, BT], BF16, tag="tm2")
                    nc.scalar.activation(
                        tm2[:], ups_tiles[i][:], AF.Identity, bias=bias_col
                    )
                    nc.vector.tensor_mul(tm3[:], tm2[:], x0T.ap()[:, dt, bs])
                else:
                    nc.vector.scalar_tensor_tensor(
                        tm3[:], ups_tiles[i][:], bias_col,
                        x0T.ap()[:, dt, bs], AO.add, AO.mult,
                    )
                adder = nc.vector if path == "a" else nc.gpsimd
                if l < L - 1:
                    adder.tensor_add(
                        xlT.ap()[:, dt, bs], xlT.ap()[:, dt, bs], tm3[:]
                    )
                else:
                    adder.tensor_add(ot[:, dt], xlT.ap()[:, dt, bs], tm3[:])
            if ot is not None:
                nc.sync.dma_start(
                    OT.ap()[:, half * (NK // 2) : (half + 1) * (NK // 2), bs],
                    ot[:, half * (NK // 2) : (half + 1) * (NK // 2)],
                )

        def body(_iv=None):
            prev = None  # (slot j-1's index, t2s tiles)
            for j in range(NSLOT):
                l, bt = divmod(j, NBT)
                bs = bass.ts(bt, BT)
                # ---- gates logits ----
                gps = ps_gs.tile([E, BT], F32, tag="gs")
                for k in range(NK):
                    nc.tensor.matmul(
                        gps[:], gt_ap(k), xlT.ap()[:, k, bs],
                        start=(k == 0), stop=(k == NK - 1),
                    )
                e4 = sb_e4.tile([E, BT], BF16, tag="e4")
                nc.scalar.activation(e4[:], gps[:], AF.Exp)
                # ---- V pass, pair 0 ----
                vps0 = ps_vc.tile([128, BT], F32, tag="vc")
                for k in range(NK):
                    nc.tensor.matmul(
                        vps0[:], vt_ap(l, k, 0), xlT.ap()[:, k, bs],
                        start=(k == 0), stop=(k == NK - 1),
                    )
                t1_0 = sb_t1.tile([128, BT], BF16, tag="t1")
                nc.scalar.activation(t1_0[:], vps0[:], AF.Tanh)
                # ---- softmax denominator + normalize (in [4, b] space) ----
                sps = ps_gs.tile([E, BT], F32, tag="gs")
                nc.tensor.matmul(sps[:], rps_ap, e4[:], start=True, stop=True)
                rs = sb_rs.tile([E, BT], F32, tag="rs")
                nc.vector.reciprocal_approx_fast(rs[:], sps[:])
                g4 = sb_g4.tile([E, BT], BF16, tag="g4")
                nc.vector.tensor_mul(g4[:], e4[:], rs[:])
                # ---- V pass, pair 1 ----
                vps1 = ps_vc.tile([128, BT], F32, tag="vc")
                for k in range(NK):
                    nc.tensor.matmul(
                        vps1[:], vt_ap(l, k, 1), xlT.ap()[:, k, bs],
                        start=(k == 0), stop=(k == NK - 1),
                    )
                t1_1 = sb_t1.tile([128, BT], BF16, tag="t1")
                nc.scalar.activation(t1_1[:], vps1[:], AF.Tanh)
                # ---- C pass, pair 0; replication rides the idle gates bank ----
                cps0 = ps_vc.tile([128, BT], F32, tag="vc")
                nc.tensor.matmul(cps0[:], cb_ap(l, 0), t1_0[:], start=True, stop=True)
                t2_0 = sb_t2.tile([128, BT], BF16, tag="t2")
                nc.scalar.activation(t2_0[:], cps0[:], AF.Tanh)
                eps0 = ps_gs.tile([128, BT], F32, tag="gs")
                nc.tensor.matmul(eps0[:], rpr_ap(0), g4[:], start=True, stop=True)
                t2s0 = sb_t2s.tile([128, BT], BF16, tag="t2s0")
                nc.vector.tensor_mul(t2s0[:], t2_0[:], eps0[:])
                # ---- U pass of the PREVIOUS slot (first half) ----
                pot = None
                if prev is not None:
                    pj, pt2s = prev
                    if pj // NBT == L - 1:
                        pot = sb_ot.tile([128, NK, BT], BF16, tag="ot")
                    ups = emit_U_half(pj, pt2s, 0)
                    emit_epilogue_half(pj, ups, 0, pot)
                # ---- C pass + replication, pair 1 ----
                cps1 = ps_vc.tile([128, BT], F32, tag="vc")
                nc.tensor.matmul(cps1[:], cb_ap(l, 1), t1_1[:], start=True, stop=True)
                t2_1 = sb_t2.tile([128, BT], BF16, tag="t2")
                nc.scalar.activation(t2_1[:], cps1[:], AF.Tanh)
                eps1 = ps_e.tile([128, BT], F32, tag="e")
                nc.tensor.matmul(eps1[:], rpr_ap(1), g4[:], start=True, stop=True)
                # ---- U pass of the PREVIOUS slot (second half) ----
                if prev is not None:
                    ups = emit_U_half(pj, pt2s, 1)
                    emit_epilogue_half(pj, ups, 1, pot)
                # ---- gate scaling, pair 1 ----
                t2s1 = sb_t2s.tile([128, BT], BF16, tag="t2s1")
                nc.vector.tensor_mul(t2s1[:], t2_1[:], eps1[:])
                prev = (j, (t2s0, t2s1))
            # drain the pipeline: last slot's U + epilogue
            pj, pt2s = prev
            pot = sb_ot.tile([128, NK, BT], BF16, tag="ot")
            for half in range(2):
                ups = emit_U_half(pj, pt2s, half, drain=True)
                emit_epilogue_half(pj, ups, half, pot)

        if repeat == 1:
            body()
        else:
            with tc.For_i(0, repeat, 1) as _i:
                body(_i)
        if bench:
            dtile = sb_tm.tile([1, 1], F32, tag="dummy")
            nc.sync.dma_start(dtile[:], dummy_in.ap())
            nc.sync.dma_start(dummy_out.ap(), dtile[:])
        ctx.close()

    nc.compile()
    _cache[key] = nc
    return nc


def _bf16(a):
    return np.ascontiguousarray(a).astype(ml_dtypes.bfloat16)


def _prep(x, U, V, C, bias, gates_w):
    """Host-side layout prep. Returns list of per-core input dicts."""
    x = np.ascontiguousarray(x, dtype=np.float32)
    # Vt[l, d, e*R+r] = V[l, e, r, d]; partition-major chunks of d.
    Vt = (
        V.astype(np.float32)
        .transpose(0, 3, 1, 2)
        .reshape(L, NK, 128, E * R)
        .transpose(0, 2, 1, 3)          # [L, 128, NK, 256]
        .reshape(L, 128, NK * 256)
    )
    Gt = (
        gates_w.astype(np.float32)
        .T.reshape(NK, 128, E)
        .transpose(1, 0, 2)
        .reshape(128, NK * E)
    )
    Cbd = np.zeros((L, 2, 128, 128), dtype=np.float32)
    for l in range(L):
        for p in range(2):
            Cbd[l, p, :R, :R] = C[l, 2 * p].T
            Cbd[l, p, R:, R:] = C[l, 2 * p + 1].T
    Cbp = Cbd.transpose(0, 2, 1, 3).reshape(L, 128, 2 * 128)
    Ut = np.zeros((L, 2, 128, D), dtype=np.float32)
    for l in range(L):
        for p in range(2):
            Ut[l, p, :R] = U[l, 2 * p].T
            Ut[l, p, R:] = U[l, 2 * p + 1].T
    Utp = Ut.transpose(0, 2, 1, 3).reshape(L, 128, 2 * D)
    pk0 = np.concatenate([Gt, Cbp[0], Vt[0]], axis=1)
    pk12 = np.concatenate(
        [np.concatenate([Cbp[l], Vt[l], Utp[l]], axis=1) for l in (1, 2)], axis=1
    )
    rr = np.zeros((E, E + 2 * 128), dtype=np.float32)
    rr[:, :E] = 1.0
    for p in range(2):
        for m in range(128):
            rr[2 * p + m // 64, E + p * 128 + m] = 1.0
    BTb = np.ascontiguousarray(
        bias.astype(np.float32).reshape(L, NK, 128).transpose(2, 0, 1).reshape(128, L * NK)
    )
    shared = {
        "PK0": _bf16(pk0), "UT0": _bf16(Utp[0]), "PK12": _bf16(pk12),
        "RR": _bf16(rr), "BTb": BTb,
    }
    per_core = []
    for i in range(NCORES):
        xTi = x[i * BC : (i + 1) * BC].T          # [D, BC]
        xTp = xTi.reshape(NK, 128, BC).transpose(1, 0, 2)   # [128, NK, BC]
        per_core.append({"XT": _bf16(xTp), **shared})
    return per_core


def kernel(x, U, V, C, bias, gates_w):
    nc = _build(1)
    in_maps = _prep(x, U, V, C, bias, gates_w)
    res = run_bass_kernel_spmd(nc, in_maps, list(range(NCORES)))
    out = np.empty((B, D), dtype=np.float32)
    for i in range(NCORES):
        o = np.asarray(res.results[i]["OT"]).astype(np.float32)  # [128, NK, BC]
        out[i * BC : (i + 1) * BC] = o.transpose(1, 0, 2).reshape(D, BC).T
    return out


if __name__ == "__main__":
    rng = np.random.default_rng(0)
    x = rng.standard_normal((B, D), dtype=np.float32)
    su = (2.0 / (D + R)) ** 0.5
    sc = (2.0 / (R + R)) ** 0.5
    U_ = rng.standard_normal((L, E, D, R), dtype=np.float32) * su
    V_ = rng.standard_normal((L, E, R, D), dtype=np.float32) * su
    C_ = rng.standard_normal((L, E, R, R), dtype=np.float32) * sc
    b_ = np.zeros((L, D), dtype=np.float32)
    g_ = rng.standard_normal((E, D), dtype=np.float32) / np.sqrt(D)
    out = kernel(x, U_, V_, C_, b_, g_)

    # numpy reference
    x0, xl = x, x.astype(np.float64)
    for i in range(L):
        logits = xl @ g_.T.astype(np.float64)
        ex = np.exp(logits - logits.max(axis=1, keepdims=True))
        g = ex / ex.sum(axis=1, keepdims=True)
        t = np.tanh(np.einsum("erd,bd->ber", V_[i].astype(np.float64), xl))
        t = np.tanh(np.einsum("ers,bes->ber", C_[i].astype(np.float64), t))
        t = np.einsum("edr,ber->bed", U_[i].astype(np.float64), t) + b_[i][None, None, :]
        t = x0[:, None, :] * t
        xl = np.einsum("bed,be->bd", t, g) + xl
    err = np.abs(out - xl)
    print(f"absmax={err.max():.4e} rel={err.max()/np.abs(xl).max():.4e}")

